# revision 6
# baseline (speedup 1.0000x reference)
"""Paged GQA decode attention (B=64, HQ=32, HKV=8, D=128) on 8 TRN2 NeuronCores.

Strategy: flat chunk-parallel SPMD.
 - Every request is cut into 128-token chunks (533 total for this seed); the
   flat chunk list is split evenly across the 8 cores (padded to a DMA-group
   multiple), so all cores stream the same byte count and run one program.
 - No softmax-max pass: scores are shifted by a fixed VSHIFT and masked with
   an additive bias (0/-30), so partial (numerator, denominator) sums over
   disjoint token sets simply add — the host merges per-request partials.
 - Host gathers each chunk's KV blocks (honoring block_tables) into one
   contiguous stream per core: K pre-transposed to [d, token] tiles, V
   natural [token, d], both bf16, packed K|V into one [128, 8K] group tile
   per GRP=4 chunks = one 2 MB HWDGE DMA (nc.sync) per group.
 - Per chunk on device: scores[tok, hq] = K_h^T.T @ qT_c (8 matmuls into
   PSUM), E = exp(scores + bias_c) on ScalarE (bias masks invalid/padded
   tokens), then PV: acc[d, 4] = V_h.T @ E_h (8 matmuls, start/stop=True)
   into per-chunk columns of a PSUM bank that holds 16 chunks' partials,
   plus a ones-matmul denominator per chunk. Every 16 chunks the bank is
   copied to SBUF (DVE) and DMA'd out (gpsimd). Final division on host.
"""

import math
import os
import sys
from contextlib import ExitStack

import numpy as np
import ml_dtypes  # noqa: F401  (numpy bf16 dtype)

for _p in ("/opt/trn_rl_repo", "/root/.axon_site/_ro/trn_rl_repo"):
    if os.path.isdir(_p) and _p not in sys.path:
        sys.path.insert(0, _p)
        break

import concourse.bass as bass  # noqa: F401
import concourse.tile as tile
from concourse import bacc, mybir
from concourse.bass_utils import run_bass_kernel_spmd

B, HQ, HKV, D, BS, MB = 64, 32, 8, 128, 16, 128
G = HQ // HKV              # 4 query heads per kv head
SCALE = 0.08838834764831845
NCORES = 8
CHUNK = 128                # tokens per chunk (= SBUF partitions)
BPC = CHUNK // BS          # blocks per chunk = 8
ROW = HKV * D              # 1024 elements per token row
NEG = -30.0                # additive mask for invalid tokens
VSHIFT = -2.0              # fixed score shift (replaces softmax max pass)
GRP = 4                    # chunks per DMA group (one combined K|V transfer)
PGRP = 16                  # chunks per PSUM accumulation bank (32 cols each)
KV_BUFS = 6                # group tiles in flight (2 MB each)
KV_ENG = "sync"            # engine issuing the K|V group DMAs (HWDGE ring 1)
IN_ENG = "scalar"          # engine issuing qc/bias DMAs (HWDGE ring 2)
OUT_ENG = "gpsimd"         # engine issuing staging/output DMAs (SWDGE)

last_results = None        # stashed BassKernelResults for test.py

_prog_cache = {}

_bf16 = mybir.dt.bfloat16
_f32 = mybir.dt.float32
np_bf16 = mybir.dt.np(_bf16)


def _build_program(C):
    """C = chunks per core (multiple of GRP)."""
    NG = C // GRP
    NP = (C + PGRP - 1) // PGRP
    nc = bacc.Bacc()

    kv_d = nc.declare_dram_parameter("kv", [NG, CHUNK, 2 * GRP * ROW], _bf16,
                                     isOutput=False)
    qc_d = nc.declare_dram_parameter("qc", [D, C * HQ], _bf16, isOutput=False)
    bias_d = nc.declare_dram_parameter("bias", [CHUNK, C], _f32,
                                       isOutput=False)
    acc_d = nc.declare_dram_parameter("acc", [NP, D, PGRP * HQ], _bf16,
                                      isOutput=True)
    den_d = nc.declare_dram_parameter("den", [HQ, C], _f32, isOutput=True)

    EXP = mybir.ActivationFunctionType.Exp

    with tile.TileContext(nc) as tc, ExitStack() as ctx:
        kvpool = ctx.enter_context(tc.tile_pool(name="kv", bufs=KV_BUFS))
        epool = ctx.enter_context(tc.tile_pool(name="e", bufs=3))
        stage = ctx.enter_context(tc.tile_pool(name="st", bufs=2))
        const = ctx.enter_context(tc.tile_pool(name="cst", bufs=1))
        spsum = ctx.enter_context(tc.tile_pool(name="sp", bufs=2, space="PSUM"))
        apsum = ctx.enter_context(tc.tile_pool(name="ac", bufs=2, space="PSUM"))
        dpsum = ctx.enter_context(tc.tile_pool(name="dp", bufs=2, space="PSUM"))

        bias_t = const.tile([CHUNK, C], _f32)
        getattr(nc, IN_ENG).dma_start(bias_t[:], bias_d[:])
        q_all = const.tile([D, C * HQ], _bf16)
        getattr(nc, IN_ENG).dma_start(q_all[:], qc_d[:])
        den_s = const.tile([HQ, C], _f32)
        # ones on ScalarE so the denominator matmul's deps stay in the single
        # ACT semaphore domain (PE matmuls support only one sync wait).
        ones = const.tile([CHUNK, 1], _bf16)
        nc.scalar.activation(ones[:], bias_t[:, 0:1],
                             mybir.ActivationFunctionType.Identity,
                             bias=1.0, scale=0.0)
        # dummy matmul absorbs the q DMA wait so the first real matmul
        # only waits on its kv DMA.
        dmy = spsum.tile([1, 1], _f32, tag="sco")
        nc.tensor.matmul(dmy[:], q_all[0:1, 0:1], q_all[0:1, 0:1],
                         start=True, stop=True)

        kvtiles = {}
        scos = {}

        def emit_sco(c):
            """Scores for chunk c; issues the group DMA on its first chunk.
            Emitted one chunk ahead of emit_pv(c) so the PE computes chunk
            c+1's scores while ScalarE runs exp(c) — no per-chunk PE stall."""
            g, half = divmod(c, GRP)
            if half == 0:
                kvt = kvpool.tile([CHUNK, 2 * GRP * ROW], _bf16, tag="kv",
                                  name="kv")
                getattr(nc, KV_ENG).dma_start(kvt[:], kv_d[g])
                kvtiles[g] = kvt
            kt = kvtiles[g][:, half * ROW:(half + 1) * ROW]
            sco = spsum.tile([CHUNK, HQ], _f32, tag="sco")
            for h in range(HKV):
                nc.tensor.matmul(
                    sco[:, h * G:(h + 1) * G],
                    kt[:, h * D:(h + 1) * D],
                    q_all[:, c * HQ + h * G:c * HQ + (h + 1) * G],
                    start=True, stop=True,
                )
            scos[c] = sco

        accg = deng = None
        emit_sco(0)
        for c in range(C):
            g, half = divmod(c, GRP)
            et = epool.tile([CHUNK, HQ], _bf16)
            nc.scalar.activation(et[:], scos.pop(c)[:], EXP,
                                 bias=bias_t[:, c:c + 1], scale=1.0)
            if c + 1 < C:
                emit_sco(c + 1)

            vt = kvtiles[g][:, (GRP + half) * ROW:(GRP + half + 1) * ROW]
            jm = c % PGRP
            if jm == 0:
                accg = apsum.tile([D, PGRP * HQ], _f32, tag="acc")
                deng = dpsum.tile([HQ, PGRP], _f32, tag="den")
            for h in range(HKV):
                nc.tensor.matmul(
                    accg[:, jm * HQ + h * G:jm * HQ + (h + 1) * G],
                    vt[:, h * D:(h + 1) * D],
                    et[:, h * G:(h + 1) * G],
                    start=True, stop=True,
                )
            nc.tensor.matmul(deng[:, jm:jm + 1], et[:], ones[:],
                             start=True, stop=True)

            if jm == PGRP - 1 or c == C - 1:
                p = c // PGRP
                used = (jm + 1) * HQ
                sacc = stage.tile([D, PGRP * HQ], _bf16, tag="sacc")
                nc.vector.tensor_copy(sacc[:, :used], accg[:, :used])
                getattr(nc, OUT_ENG).dma_start(acc_d[p], sacc[:])
                nc.vector.tensor_copy(den_s[:, p * PGRP:p * PGRP + jm + 1],
                                      deng[:, :jm + 1])
        getattr(nc, OUT_ENG).dma_start(den_d[:], den_s[:])
    nc.compile()
    return nc


def _get_program(C):
    if C not in _prog_cache:
        _prog_cache[C] = _build_program(C)
    return _prog_cache[C]


def _make_schedule(context_lens):
    """Flat chunk list → per-core spans of C chunks each."""
    L = context_lens.astype(np.int64)
    chunks = []  # (request, chunk_idx)
    for b in range(B):
        for j in range(max(1, math.ceil(int(L[b]) / CHUNK))):
            chunks.append((b, j))
    C = math.ceil(len(chunks) / NCORES)
    C = math.ceil(C / GRP) * GRP
    pad = C * NCORES - len(chunks)
    chunks += [(-1, 0)] * pad
    return chunks, C


def _build_in_maps(q, k_cache, v_cache, block_tables, L, chunks, C):
    NG = C // GRP
    nblocks_total = k_cache.shape[0]
    kf = k_cache.reshape(nblocks_total, BS, ROW)
    vf = v_cache.reshape(nblocks_total, BS, ROW)
    qT = np.empty((B, D, HQ), np_bf16)
    for b in range(B):
        qT[b] = (q[b] * SCALE).T
    tok = np.arange(CHUNK, dtype=np.int64)

    in_maps = []
    for cidx in range(NCORES):
        span = chunks[cidx * C:(cidx + 1) * C]
        karr = np.zeros((C, D, ROW), np_bf16)
        varr = np.zeros((C, CHUNK, ROW), np_bf16)
        biasT = np.full((C, CHUNK), NEG, np.float32)
        qc = np.zeros((D, C * HQ), np_bf16)
        for i, (b, j) in enumerate(span):
            if b < 0:
                continue
            blocks = np.clip(
                block_tables[b, j * BPC:(j + 1) * BPC].astype(np.int64),
                0, nblocks_total - 1)
            kreq = kf[blocks].reshape(CHUNK, HKV, D)
            karr[i] = kreq.transpose(2, 1, 0).reshape(D, ROW)
            varr[i] = vf[blocks].reshape(CHUNK, ROW)
            biasT[i] = np.where(j * CHUNK + tok < int(L[b]), VSHIFT, NEG)
            qc[:, i * HQ:(i + 1) * HQ] = qT[b]
        kg = np.ascontiguousarray(
            karr.reshape(NG, GRP, D, ROW).transpose(0, 2, 1, 3)
        ).reshape(NG, D, GRP * ROW)
        vg = np.ascontiguousarray(
            varr.reshape(NG, GRP, CHUNK, ROW).transpose(0, 2, 1, 3)
        ).reshape(NG, CHUNK, GRP * ROW)
        in_maps.append({
            "kv": np.concatenate([kg, vg], axis=2),
            "qc": qc,
            "bias": np.ascontiguousarray(biasT.T),
        })
    return in_maps


def kernel(q, k_cache, v_cache, block_tables, context_lens):
    global last_results
    q = np.asarray(q, dtype=np.float32)
    k_cache = np.asarray(k_cache, dtype=np.float32)
    v_cache = np.asarray(v_cache, dtype=np.float32)
    block_tables = np.asarray(block_tables, dtype=np.int32)
    context_lens = np.asarray(context_lens, dtype=np.int32)

    L = context_lens.astype(np.int64)
    chunks, C = _make_schedule(context_lens)
    nc = _get_program(C)
    in_maps = _build_in_maps(q, k_cache, v_cache, block_tables, L, chunks, C)

    res = run_bass_kernel_spmd(
        nc, in_maps, list(range(NCORES)),
        trace=bool(os.environ.get("KBASS_TRACE")),
    )
    last_results = res

    NP = (C + PGRP - 1) // PGRP
    acc = np.zeros((B, HQ, D), np.float64)
    den = np.zeros((B, HQ), np.float64)
    for cidx in range(NCORES):
        # [NP, D, PGRP*HQ] -> per-chunk [HQ, D]
        pacc = res.results[cidx]["acc"].astype(np.float32) \
            .reshape(NP, D, PGRP, HQ) \
            .transpose(0, 2, 3, 1).reshape(NP * PGRP, HQ, D)
        pden = res.results[cidx]["den"]  # [HQ, C]
        for i, (b, j) in enumerate(chunks[cidx * C:(cidx + 1) * C]):
            if b < 0:
                continue
            acc[b] += pacc[i]
            den[b] += pden[:, i]
    out = acc / np.maximum(den, 1e-30)[:, :, None]
    return out.astype(np.float32)


# revision 14
# speedup vs baseline: 1.1335x; 1.1335x over previous
"""Paged GQA decode attention (B=64, HQ=32, HKV=8, D=128) on 8 TRN2 NeuronCores.

Strategy: flat chunk-parallel SPMD.
 - Every request is cut into 128-token chunks (533 total for this seed); the
   flat chunk list is split evenly across the 8 cores (padded to a DMA-group
   multiple), so all cores stream the same byte count and run one program.
 - No softmax-max pass: scores are shifted by a fixed VSHIFT and masked with
   an additive bias (0/-30), so partial (numerator, denominator) sums over
   disjoint token sets simply add — the host merges per-request partials.
 - Host gathers each chunk's KV blocks (honoring block_tables) into one
   contiguous stream per core: K pre-transposed to [d, token] tiles, V
   natural [token, d], both bf16, packed K|V into one [128, 8K] group tile
   per GRP=4 chunks = one 2 MB HWDGE DMA (nc.sync) per group.
 - Per chunk on device: scores[tok, hq] = K_h^T.T @ qT_c (8 matmuls into
   PSUM), E = exp(scores + bias_c) on ScalarE (bias masks invalid/padded
   tokens), then PV: acc[d, 4] = V_h.T @ E_h (8 matmuls, start/stop=True)
   into per-chunk columns of a PSUM bank that holds 16 chunks' partials,
   plus a ones-matmul denominator per chunk. Every 16 chunks the bank is
   copied to SBUF (DVE) and DMA'd out (gpsimd). Final division on host.
"""

import math
import os
import sys
from contextlib import ExitStack

import numpy as np
import ml_dtypes  # noqa: F401  (numpy bf16 dtype)

for _p in ("/opt/trn_rl_repo", "/root/.axon_site/_ro/trn_rl_repo"):
    if os.path.isdir(_p) and _p not in sys.path:
        sys.path.insert(0, _p)
        break

import concourse.bass as bass  # noqa: F401
import concourse.tile as tile
from concourse import bacc, mybir
from concourse.bass_utils import run_bass_kernel_spmd

B, HQ, HKV, D, BS, MB = 64, 32, 8, 128, 16, 128
G = HQ // HKV              # 4 query heads per kv head
SCALE = 0.08838834764831845
NCORES = 8
CHUNK = 128                # tokens per chunk (= SBUF partitions)
BPC = CHUNK // BS          # blocks per chunk = 8
ROW = HKV * D              # 1024 elements per token row
NEG = -30.0                # additive mask for invalid tokens
VSHIFT = -2.0              # fixed score shift (replaces softmax max pass)
GRP = 4                    # chunks per DMA group (one combined K|V transfer)
PGRP = 16                  # chunks per PSUM accumulation bank (32 cols each)
KV_BUFS = 8                # group tiles in flight (2 MB each)
KV_ENG = "sync"            # engine issuing the K|V group DMAs (HWDGE ring 1)
IN_ENG = "gpsimd"          # engine issuing qc/bias DMAs (SWDGE, off hot path)
OUT_ENG = "gpsimd"         # engine issuing mid-run staging DMAs (SWDGE)

last_results = None        # stashed BassKernelResults for test.py

_prog_cache = {}

_bf16 = mybir.dt.bfloat16
_f32 = mybir.dt.float32
np_bf16 = mybir.dt.np(_bf16)


def _build_program(C):
    """C = chunks per core (last DMA group may be partial)."""
    NG = (C + GRP - 1) // GRP
    NP = (C + PGRP - 1) // PGRP
    nc = bacc.Bacc()

    kv_d = nc.declare_dram_parameter("kv", [CHUNK, C * 2 * ROW], _bf16,
                                     isOutput=False)
    qc_d = nc.declare_dram_parameter("qc", [D, C * HQ], _bf16, isOutput=False)
    bias_d = nc.declare_dram_parameter("bias", [CHUNK, C], _f32,
                                       isOutput=False)
    acc_d = nc.declare_dram_parameter("acc", [NP, D, PGRP * HQ], _bf16,
                                      isOutput=True)
    den_d = nc.declare_dram_parameter("den", [HQ, C], _f32, isOutput=True)

    EXP = mybir.ActivationFunctionType.Exp

    with tile.TileContext(nc) as tc, ExitStack() as ctx:
        kvpool = ctx.enter_context(tc.tile_pool(name="kv", bufs=KV_BUFS))
        epool = ctx.enter_context(tc.tile_pool(name="e", bufs=3))
        stage = ctx.enter_context(tc.tile_pool(name="st", bufs=2))
        const = ctx.enter_context(tc.tile_pool(name="cst", bufs=1))
        spsum = ctx.enter_context(tc.tile_pool(name="sp", bufs=2, space="PSUM"))
        apsum = ctx.enter_context(tc.tile_pool(name="ac", bufs=2, space="PSUM"))
        dpsum = ctx.enter_context(tc.tile_pool(name="dp", bufs=2, space="PSUM"))

        bias_t = const.tile([CHUNK, C], _f32)
        getattr(nc, IN_ENG).dma_start(bias_t[:], bias_d[:])
        q_all = const.tile([D, C * HQ], _bf16)
        getattr(nc, IN_ENG).dma_start(q_all[:], qc_d[:])
        den_s = const.tile([HQ, C], _f32)
        ones = const.tile([CHUNK, 1], _bf16)

        kvtiles = {}
        scos = {}

        def emit_sco(c):
            """Scores for chunk c; issues the group DMA on its first chunk.
            Emitted one chunk ahead of the PV stage so the PE computes chunk
            c+1's scores while ScalarE runs exp(c) — no per-chunk PE stall."""
            g, half = divmod(c, GRP)
            glen = min(GRP, C - g * GRP)
            if half == 0:
                kvt = kvpool.tile([CHUNK, 2 * GRP * ROW], _bf16, tag="kv",
                                  name="kv")
                off = g * GRP * 2 * ROW
                # group 0 on the second HWDGE ring so both rings emit their
                # first descriptors concurrently at startup
                eng = nc.scalar if g == 0 else getattr(nc, KV_ENG)
                eng.dma_start(kvt[:, :2 * glen * ROW],
                              kv_d[:, off:off + 2 * glen * ROW])
                kvtiles[g] = (kvt, glen)
            kvt, glen = kvtiles[g]
            kt = kvt[:, half * ROW:(half + 1) * ROW]
            sco = spsum.tile([CHUNK, HQ], _f32, tag="sco")
            for h in range(HKV):
                nc.tensor.matmul(
                    sco[:, h * G:(h + 1) * G],
                    kt[:, h * D:(h + 1) * D],
                    q_all[:, c * HQ + h * G:c * HQ + (h + 1) * G],
                    start=True, stop=True,
                )
            scos[c] = sco

        # dummy matmul absorbs the q DMA wait so the first real matmul
        # only waits on its kv DMA.
        dmy = spsum.tile([1, 1], _f32, tag="sco")
        nc.tensor.matmul(dmy[:], q_all[0:1, 0:1], q_all[0:1, 0:1],
                         start=True, stop=True)
        accg = deng = None
        emit_sco(0)  # also issues the group-0 DMA as ACT's first instruction
        # ones on ScalarE so the denominator matmul's deps stay in the single
        # ACT semaphore domain (PE matmuls support only one sync wait).
        nc.scalar.activation(ones[:], bias_t[:, 0:1],
                             mybir.ActivationFunctionType.Identity,
                             bias=1.0, scale=0.0)
        for c in range(C):
            g, half = divmod(c, GRP)
            et = epool.tile([CHUNK, HQ], _bf16)
            nc.scalar.activation(et[:], scos.pop(c)[:], EXP,
                                 bias=bias_t[:, c:c + 1], scale=1.0)
            if c + 1 < C:
                emit_sco(c + 1)

            kvt, glen = kvtiles[g]
            vt = kvt[:, (glen + half) * ROW:(glen + half + 1) * ROW]
            jm = c % PGRP
            if jm == 0:
                accg = apsum.tile([D, PGRP * HQ], _f32, tag="acc")
                deng = dpsum.tile([HQ, PGRP], _f32, tag="den")
            for h in range(HKV):
                nc.tensor.matmul(
                    accg[:, jm * HQ + h * G:jm * HQ + (h + 1) * G],
                    vt[:, h * D:(h + 1) * D],
                    et[:, h * G:(h + 1) * G],
                    start=True, stop=True,
                )
            nc.tensor.matmul(deng[:, jm:jm + 1], et[:], ones[:],
                             start=True, stop=True)

            if jm == PGRP - 1 or c == C - 1:
                p = c // PGRP
                used = (jm + 1) * HQ
                last = c == C - 1
                sacc = stage.tile([D, PGRP * HQ], _bf16, tag="sacc")
                nc.vector.tensor_copy(sacc[:, :used], accg[:, :used])
                # final flushes ride the idle HWDGE rings (no Q7 emission
                # latency at the tail); mid-run flushes stay on SWDGE
                aeng = getattr(nc, KV_ENG) if last else getattr(nc, OUT_ENG)
                aeng.dma_start(acc_d[p], sacc[:])
                nc.vector.tensor_copy(den_s[:, p * PGRP:p * PGRP + jm + 1],
                                      deng[:, :jm + 1])
        nc.scalar.dma_start(den_d[:], den_s[:])
    nc.compile()
    return nc


def _get_program(C):
    if C not in _prog_cache:
        _prog_cache[C] = _build_program(C)
    return _prog_cache[C]


def _make_schedule(context_lens):
    """Flat chunk list → per-core spans of C chunks each."""
    L = context_lens.astype(np.int64)
    chunks = []  # (request, chunk_idx)
    for b in range(B):
        for j in range(max(1, math.ceil(int(L[b]) / CHUNK))):
            chunks.append((b, j))
    C = math.ceil(len(chunks) / NCORES)
    pad = C * NCORES - len(chunks)
    chunks += [(-1, 0)] * pad
    return chunks, C


def _build_in_maps(q, k_cache, v_cache, block_tables, L, chunks, C):
    nblocks_total = k_cache.shape[0]
    kf = k_cache.reshape(nblocks_total, BS, ROW)
    vf = v_cache.reshape(nblocks_total, BS, ROW)
    qT = np.empty((B, D, HQ), np_bf16)
    for b in range(B):
        qT[b] = (q[b] * SCALE).T
    tok = np.arange(CHUNK, dtype=np.int64)

    in_maps = []
    for cidx in range(NCORES):
        span = chunks[cidx * C:(cidx + 1) * C]
        karr = np.zeros((C, D, ROW), np_bf16)
        varr = np.zeros((C, CHUNK, ROW), np_bf16)
        biasT = np.full((C, CHUNK), NEG, np.float32)
        qc = np.zeros((D, C * HQ), np_bf16)
        for i, (b, j) in enumerate(span):
            if b < 0:
                continue
            blocks = np.clip(
                block_tables[b, j * BPC:(j + 1) * BPC].astype(np.int64),
                0, nblocks_total - 1)
            kreq = kf[blocks].reshape(CHUNK, HKV, D)
            karr[i] = kreq.transpose(2, 1, 0).reshape(D, ROW)
            varr[i] = vf[blocks].reshape(CHUNK, ROW)
            biasT[i] = np.where(j * CHUNK + tok < int(L[b]), VSHIFT, NEG)
            qc[:, i * HQ:(i + 1) * HQ] = qT[b]
        # flat [CHUNK, C*2*ROW]: per group g of glen chunks, cols are
        # [K(chunk0..glen-1) | V(chunk0..glen-1)], each chunk ROW wide
        blocks_cols = []
        for g0 in range(0, C, GRP):
            glen = min(GRP, C - g0)
            kgrp = karr[g0:g0 + glen].transpose(1, 0, 2).reshape(D, glen * ROW)
            vgrp = varr[g0:g0 + glen].transpose(1, 0, 2).reshape(CHUNK,
                                                                 glen * ROW)
            blocks_cols.append(kgrp)
            blocks_cols.append(vgrp)
        in_maps.append({
            "kv": np.ascontiguousarray(np.concatenate(blocks_cols, axis=1)),
            "qc": qc,
            "bias": np.ascontiguousarray(biasT.T),
        })
    return in_maps


def kernel(q, k_cache, v_cache, block_tables, context_lens):
    global last_results
    q = np.asarray(q, dtype=np.float32)
    k_cache = np.asarray(k_cache, dtype=np.float32)
    v_cache = np.asarray(v_cache, dtype=np.float32)
    block_tables = np.asarray(block_tables, dtype=np.int32)
    context_lens = np.asarray(context_lens, dtype=np.int32)

    L = context_lens.astype(np.int64)
    chunks, C = _make_schedule(context_lens)
    nc = _get_program(C)
    in_maps = _build_in_maps(q, k_cache, v_cache, block_tables, L, chunks, C)

    res = run_bass_kernel_spmd(
        nc, in_maps, list(range(NCORES)),
        trace=bool(os.environ.get("KBASS_TRACE")),
    )
    last_results = res

    NP = (C + PGRP - 1) // PGRP
    acc = np.zeros((B, HQ, D), np.float64)
    den = np.zeros((B, HQ), np.float64)
    for cidx in range(NCORES):
        # [NP, D, PGRP*HQ] -> per-chunk [HQ, D]
        pacc = res.results[cidx]["acc"].astype(np.float32) \
            .reshape(NP, D, PGRP, HQ) \
            .transpose(0, 2, 3, 1).reshape(NP * PGRP, HQ, D)
        pden = res.results[cidx]["den"]  # [HQ, C]
        for i, (b, j) in enumerate(chunks[cidx * C:(cidx + 1) * C]):
            if b < 0:
                continue
            acc[b] += pacc[i]
            den[b] += pden[:, i]
    out = acc / np.maximum(den, 1e-30)[:, :, None]
    return out.astype(np.float32)
